# revision 8
# baseline (speedup 1.0000x reference)
"""Trainium2 Bass kernel for a 3-layer GCN (KnowledgeGraphGNN).

Reference (per layer i):  h = BN_i(relu(adj @ (h @ W_i) + b_i)),  then
out = h @ Wout + bout.

Sharding: nodes row-partitioned over 8 cores.  Each core keeps its adj^T
column block [N, R] resident in SBUF as fp8 (adj is 0/1 -> exact) and the
full "stationary" activation matrix [N, 128] in bf16 k-tiles.  The
aggregation matmul runs in transposed space: P^T [128, R] = S^T @ adjT_c,
64 k-tiles accumulated on the PE (N=512 moving slices).

Layer 0 uses associativity: adj @ (x W0) = (adj @ x) @ W0, so the
stationaries are x's own k-tiles (DMA'd in natural layout, no on-chip
build) and W0 is applied post-aggregation like the BN-folded W_i of later
layers.  Layer-0 accumulation is group-major so the PE tracks the
streaming adj DMA.

Collective structure: each hidden layer's AllGather of the raw post-ReLU
activations R_i is SPLIT in two half-payload AllGathers (chunk 0 rows,
then chunk 1 rows + packed BN partial sums).  AG_a's wire time hides
under the producer's chunk-1 matmuls; AG_b's hides under the consumer's
first-half k-tile accumulation (the 32 k-tiles AG_a already delivered).
The BN affine is folded into the next layer algebraically:

    h_{i+1} = adj @ (BN_i(R_i) @ W) = (adj @ R_i) @ diag(a_i) W  +  d x r_i

with a_i = gamma_i/sigma_i, r_i = (beta_i - mu_i a_i) @ W, and d = adj @ 1
the degree vector.  The rank-1 d x r term is seeded into PSUM with a K=1
outer-product matmul before the W matmuls accumulate on top.  The final
layer needs only a tiny stats AllGather; its output is produced
transposed ([DOUT, R] via diag(a2)Wout-matmuls against the resident z^T)
and untransposed host-side.
"""

import numpy as np
import ml_dtypes

BF16 = ml_dtypes.bfloat16
FP8 = ml_dtypes.float8_e4m3

N = 8192          # nodes
DH = 128          # hidden dim (= partition count)
DOUT = 64
NC = 8            # cores
R = N // NC       # rows per core = 1024
KT = N // 128     # contraction k-tiles = 64
G = 8             # k-tile groups (8 tiles each)
RT = R // 128     # node tiles per core = 8
NLAYERS = 3
EPS = 1e-5
HR = 512          # half-payload rows
BROWS = HR + 4    # AG_b rows: chunk-1 rows + 4 rows (=1KB) stats

_cache = {}


def _build_module():
    from concourse import bacc, tile
    import concourse.mybir as mybir

    f32 = mybir.dt.float32
    bf16 = mybir.dt.bfloat16
    fp8 = mybir.dt.float8e4
    AF = mybir.ActivationFunctionType

    nc = bacc.Bacc(None, target_bir_lowering=False, num_devices=NC)

    # ---- kernel I/O --------------------------------------------------------
    adjt = nc.dram_tensor("adjt", [N, R], fp8, kind="ExternalInput")
    xn = nc.dram_tensor("xn", [N, 128], bf16, kind="ExternalInput")
    w0 = nc.dram_tensor("w0", [128, 128], bf16, kind="ExternalInput")
    w1 = nc.dram_tensor("w1", [128, 128], bf16, kind="ExternalInput")
    w2 = nc.dram_tensor("w2", [128, 128], bf16, kind="ExternalInput")
    wout = nc.dram_tensor("wout", [128, DOUT], f32, kind="ExternalInput")
    boutb = nc.dram_tensor("boutb", [1, DOUT], f32, kind="ExternalInput")
    dd = nc.dram_tensor("dd", [1, R], bf16, kind="ExternalInput")
    idn = nc.dram_tensor("idn", [128, 128], bf16, kind="ExternalInput")
    biasd = nc.dram_tensor("biasd", [NLAYERS, 128, 1], f32, kind="ExternalInput")
    gammad = nc.dram_tensor("gammad", [NLAYERS, 128, 1], f32, kind="ExternalInput")
    betad = nc.dram_tensor("betad", [NLAYERS, 128, 1], f32, kind="ExternalInput")
    out = nc.dram_tensor("out", [DOUT, R], f32, kind="ExternalOutput")

    rg = [list(range(NC))]
    wdram = (w0, w1, w2)

    with tile.TileContext(nc) as tc:
        with (
            tc.tile_pool(name="const", bufs=1) as const,
            tc.tile_pool(name="adjp", bufs=1) as adjp,
            tc.tile_pool(name="sp", bufs=1) as sp,
            tc.tile_pool(name="work", bufs=1) as work,
            tc.tile_pool(name="psA", bufs=1, space="PSUM") as psA,
            tc.tile_pool(name="psH", bufs=1, space="PSUM") as psH,
            tc.tile_pool(name="psS", bufs=2, space="PSUM") as psS,
            tc.tile_pool(name="psT", bufs=2, space="PSUM") as psT,
            tc.tile_pool(name="dram", bufs=1, space="DRAM") as dram,
        ):
            # ---- constants (scalar engine issues these tiny DMAs) ----------
            w_sb = []
            for i in range(NLAYERS):
                t = const.tile([128, 128], bf16, name=f"w{i}_sb", tag=f"w{i}_sb")
                nc.scalar.dma_start(t[:], wdram[i][:])
                w_sb.append(t)
            wout_sb = const.tile([128, DOUT], f32, name="wout_sb")
            nc.scalar.dma_start(wout_sb[:], wout[:])
            boutb_sb = const.tile([1, DOUT], f32, name="boutb_sb")
            nc.scalar.dma_start(boutb_sb[:], boutb[:])
            w0f = const.tile([128, 128], f32, name="w0f")
            nc.vector.tensor_copy(w0f[:], w_sb[0][:])
            ones_sb = const.tile([1, 512], bf16, name="ones_sb")
            nc.vector.memset(ones_sb[:], 1.0)
            d_sb = const.tile([1, R], bf16, name="d_sb")
            nc.scalar.dma_start(d_sb[:], dd[:])
            idn_sb = const.tile([128, 128], bf16, name="idn_sb")
            nc.scalar.dma_start(idn_sb[:], idn[:])
            bias_sb = const.tile([128, NLAYERS], f32, name="bias_sb")
            gamma_sb = const.tile([128, NLAYERS], f32, name="gamma_sb")
            beta_sb = const.tile([128, NLAYERS], f32, name="beta_sb")
            for i in range(NLAYERS):
                nc.scalar.dma_start(bias_sb[:, i : i + 1], biasd[i])
                nc.scalar.dma_start(gamma_sb[:, i : i + 1], gammad[i])
                nc.scalar.dma_start(beta_sb[:, i : i + 1], betad[i])

            # ---- stationary activation tiles (8 groups of 8 k-tiles) -------
            # Layer 0 stationaries are x's own k-tiles, DMA'd directly in
            # natural [node, feat] layout.
            s_g = [
                sp.tile([128, 8, 128], bf16, name=f"s_{g}", tag=f"s_{g}")
                for g in range(G)
            ]
            for g in range(G):
                nc.sync.dma_start(
                    s_g[g][:],
                    xn[g * 1024 : (g + 1) * 1024, :].rearrange(
                        "(k p) c -> p k c", p=128
                    ),
                )

            # ---- adj^T resident in SBUF: 8 group tiles, 1 DMA each ---------
            adj_g = []
            for g in range(G):
                t = adjp.tile([128, 8, R], fp8, name=f"adj_{g}", tag=f"adj_{g}")
                src = adjt[g * 1024 : (g + 1) * 1024, :].rearrange(
                    "(k p) c -> p k c", p=128
                )
                nc.gpsimd.dma_start(t[:], src)
                adj_g.append(t)

            def s_tile(k):
                g, sub = divmod(k, 8)
                return s_g[g][:, sub, :]

            def adj_mv(k, lo, size):
                g, sub = divmod(k, 8)
                return adj_g[g][:, sub, lo : lo + size]

            # per-layer DRAM comm tiles (split payload: a = chunk-0 rows,
            # b = chunk-1 rows + stats)
            agi = [
                dram.tile([HR + BROWS, 128], bf16, name=f"agi{i}", tag=f"agi{i}")
                for i in range(2)
            ]
            agoa = [
                dram.tile(
                    [NC * HR, 128], bf16, name=f"agoa{i}", tag=f"agoa{i}",
                    addr_space="Shared",
                )
                for i in range(2)
            ]
            agob = [
                dram.tile(
                    [NC * BROWS, 128], bf16, name=f"agob{i}", tag=f"agob{i}",
                    addr_space="Shared",
                )
                for i in range(2)
            ]
            agi2 = dram.tile([4, 128], bf16, name="agi2", tag="agi2")
            ago2 = dram.tile([32, 128], bf16, name="ago2", tag="ago2",
                             addr_space="Shared")

            gstats = None  # SBUF tile holding the 8 gathered stat blocks
            _wa = [None]
            _rrow = [None]

            def _bn_combine(i, gst, pref):
                # reduce the 8 gathered per-core stat pairs -> mu/E[x^2],
                # then inv = 1/sqrt(var+eps) with one Newton-rsqrt step to
                # scrub the HW sqrt/recip table error.
                gsc = work.tile([128, 8], f32, name=f"gsc{pref}", tag=f"gsc{pref}")
                st2 = work.tile([128, 2], f32, name=f"st2{pref}", tag=f"st2{pref}")
                nc.vector.tensor_add(gsc[:], gst[:, 0:8], gst[:, 8:16])
                nc.vector.tensor_add(gsc[:, 0:4], gsc[:, 0:4], gsc[:, 4:8])
                nc.vector.tensor_add(st2[:], gsc[:, 0:2], gsc[:, 2:4])
                mu = work.tile([128, 1], f32, name=f"mu{pref}", tag=f"mu{pref}")
                ex2 = work.tile([128, 1], f32, name=f"ex2{pref}", tag=f"ex2{pref}")
                var = work.tile([128, 1], f32, name=f"var{pref}", tag=f"var{pref}")
                sd = work.tile([128, 1], f32, name=f"sd{pref}", tag=f"sd{pref}")
                y0 = work.tile([128, 1], f32, name=f"y0{pref}", tag=f"y0{pref}")
                yy = work.tile([128, 1], f32, name=f"yy{pref}", tag=f"yy{pref}")
                hvy = work.tile([128, 1], f32, name=f"hv{pref}", tag=f"hv{pref}")
                inv = work.tile([128, 1], f32, name=f"inv{pref}", tag=f"inv{pref}")
                aco = work.tile([128, 1], f32, name=f"aco{pref}", tag=f"aco{pref}")
                cco = work.tile([128, 1], f32, name=f"cco{pref}", tag=f"cco{pref}")
                nc.vector.tensor_scalar_mul(mu[:], st2[:, 0:1], 1.0 / N)
                nc.vector.tensor_scalar_mul(ex2[:], st2[:, 1:2], 1.0 / N)
                nc.vector.tensor_mul(var[:], mu[:], mu[:])
                nc.vector.tensor_sub(var[:], ex2[:], var[:])
                nc.vector.tensor_scalar_add(var[:], var[:], EPS)
                nc.scalar.sqrt(sd[:], var[:])
                nc.vector.reciprocal(y0[:], sd[:])
                # Newton: inv = y0 * (1.5 - 0.5*var*y0^2)
                nc.vector.tensor_mul(yy[:], y0[:], y0[:])
                nc.vector.tensor_mul(yy[:], yy[:], var[:])
                nc.vector.tensor_scalar_mul(yy[:], yy[:], -0.5)
                nc.vector.tensor_scalar_add(hvy[:], yy[:], 1.5)
                nc.vector.tensor_mul(inv[:], y0[:], hvy[:])
                nc.vector.tensor_mul(aco[:], gamma_sb[:, i : i + 1], inv[:])
                nc.vector.tensor_mul(cco[:], mu[:], aco[:])
                nc.vector.tensor_sub(cco[:], beta_sb[:, i : i + 1], cco[:])
                return aco, cco

            def _emit_bn_fold(i):
                # combine gathered stats of layer i-1; build Wa = diag(a) W_i
                # and r = (beta - mu a) @ W_i
                aco, cco = _bn_combine(i - 1, gstats, f"f{i}")
                ccb = work.tile([128, 1], bf16, name=f"ccb{i}", tag=f"ccb{i}")
                nc.vector.tensor_copy(ccb[:], cco[:])
                wa = work.tile([128, 128], f32, name="wa", tag="wa")
                nc.scalar.activation(wa[:], w_sb[i][:], AF.Copy, scale=aco[:])
                pr = psS.tile([1, 128], f32, name="pr", tag="psS")
                nc.tensor.matmul(pr[:], ccb[:], w_sb[i][:])
                rrow = work.tile([1, 128], bf16, name="rrow", tag="rrow")
                nc.vector.tensor_copy(rrow[:], pr[:])
                _wa[0] = wa
                _rrow[0] = rrow

            for i in range(NLAYERS):
                zb = work.tile([128, R], bf16, name="zb", tag="zb")
                sq = work.tile([128, R], f32, name="sq", tag="sq")
                st4 = work.tile([128, 4], f32, name="st4", tag="st4")
                if i < NLAYERS - 1:
                    rnat = work.tile([128, 8, 128], bf16, name="rnat", tag="rnat")

                def _bc_chunk(c):
                    # relu+bias (+stats) for 512-chunk c, then its transposes
                    lo = c * 512
                    nc.scalar.activation(
                        zb[:, lo : lo + 512],
                        ph[:, lo : lo + 512],
                        AF.Relu,
                        bias=bias_sb[:, i : i + 1],
                        scale=1.0,
                        accum_out=st4[:, 2 * c : 2 * c + 1],
                    )
                    nc.scalar.activation(
                        sq[:, lo : lo + 512],
                        zb[:, lo : lo + 512],
                        AF.Square,
                        accum_out=st4[:, 2 * c + 1 : 2 * c + 2],
                    )
                    if i < NLAYERS - 1:
                        for t in range(4 * c, 4 * c + 4):
                            ptp = psT.tile([128, 128], bf16, name="ptp", tag="psT")
                            nc.tensor.transpose(
                                ptp[:], zb[:, t * 128 : (t + 1) * 128], idn_sb[:]
                            )
                            nc.vector.tensor_copy(rnat[:, t, :], ptp[:])
                        nc.sync.dma_start(
                            agi[i][c * 512 : (c + 1) * 512, :].rearrange(
                                "(k p) c -> p k c", p=128
                            ),
                            rnat[:, 4 * c : 4 * c + 4, :],
                        )

                def _finish_chunk(nch):
                    # pm copy, (BN fold), rank-1 seed + W matmul, relu/stats
                    lo = nch * 512
                    nc.vector.tensor_copy(pm[:, lo : lo + 512], pa[:, lo : lo + 512])
                    if i == 0:
                        nc.tensor.matmul(
                            ph[:, lo : lo + 512], w0f[:], pm[:, lo : lo + 512],
                            start=True, stop=True,
                        )
                    else:
                        if nch == 0:
                            _emit_bn_fold(i)
                        nc.tensor.matmul(
                            ph[:, lo : lo + 512], _rrow[0][:],
                            d_sb[:, lo : lo + 512], start=True, stop=False,
                        )
                        nc.tensor.matmul(
                            ph[:, lo : lo + 512], _wa[0][:],
                            pm[:, lo : lo + 512], start=False, stop=True,
                        )
                    _bc_chunk(nch)

                def _launch_ag_a():
                    nc.gpsimd.collective_compute(
                        "AllGather", mybir.AluOpType.bypass, replica_groups=rg,
                        ins=[agi[i][0:HR, :].opt()], outs=[agoa[i].opt()],
                    )

                def _launch_ag_b():
                    nc.scalar.dma_start(
                        agi[i][R : R + 4, :], st2o[:].bitcast(bf16)
                    )
                    nc.gpsimd.collective_compute(
                        "AllGather", mybir.AluOpType.bypass, replica_groups=rg,
                        ins=[agi[i][HR : HR + BROWS, :].opt()],
                        outs=[agob[i].opt()],
                    )

                pa = psA.tile([128, R], f32, name="pa", tag="pa")
                pm = work.tile([128, R], f32, name="pm", tag="pm")
                ph = psH.tile([128, R], f32, name="ph", tag="ph")
                st2o = work.tile([128, 2], f32, name="st2o", tag="st2o")

                if i == 0:
                    # group-major: chunk-0 full + half of chunk-1 per adj
                    # group (tracks the adj DMA stream), then chunk-0
                    # finishes first so AG_a launches early.
                    first = [True, True]
                    for g in range(G):
                        for sub in range(8):
                            k = 8 * g + sub
                            nc.tensor.matmul(
                                pa[:, 0:512], s_tile(k), adj_mv(k, 0, 512),
                                start=first[0], stop=(g == G - 1 and sub == 7),
                            )
                            first[0] = False
                        for sub in range(4):
                            k = 8 * g + sub
                            nc.tensor.matmul(
                                pa[:, 512:1024], s_tile(k), adj_mv(k, 512, 512),
                                start=first[1], stop=False,
                            )
                            first[1] = False
                    _finish_chunk(0)
                    _launch_ag_a()
                    for g in range(G):
                        for sub in range(4, 8):
                            k = 8 * g + sub
                            nc.tensor.matmul(
                                pa[:, 512:1024], s_tile(k), adj_mv(k, 512, 512),
                                start=False, stop=(g == G - 1 and sub == 7),
                            )
                    _finish_chunk(1)
                    nc.vector.tensor_add(st2o[:], st4[:, 0:2], st4[:, 2:4])
                    _launch_ag_b()
                else:
                    # consumer order: both chunks' first-half k-tiles (AG_a
                    # data, runs under the producer's AG_b), then chunk-0's
                    # second half -> finish chunk 0 -> AG_a out early, then
                    # chunk 1's second half -> AG_b.
                    for nch in range(2):
                        lo = nch * 512
                        for k in range(KT):
                            if k % 8 < 4:
                                nc.tensor.matmul(
                                    pa[:, lo : lo + 512], s_tile(k),
                                    adj_mv(k, lo, 512),
                                    start=(k == 0), stop=False,
                                )
                    for nch in range(2):
                        lo = nch * 512
                        for k in range(KT):
                            if k % 8 >= 4:
                                nc.tensor.matmul(
                                    pa[:, lo : lo + 512], s_tile(k),
                                    adj_mv(k, lo, 512),
                                    start=False,
                                    stop=(k == KT - 1),
                                )
                        _finish_chunk(nch)
                        if i < NLAYERS - 1:
                            if nch == 0:
                                _launch_ag_a()
                            else:
                                nc.vector.tensor_add(
                                    st2o[:], st4[:, 0:2], st4[:, 2:4]
                                )
                                _launch_ag_b()

                if i < NLAYERS - 1:
                    # unload AG_a halves on the vector queue (after AG_b's
                    # trigger so the payload path is never head-of-line
                    # blocked), AG_b halves + stats on scalar/sync.
                    for g in range(G):
                        nc.gpsimd.dma_start(
                            s_g[g][:, 0:4, :],
                            agoa[i][g * HR : (g + 1) * HR, :].rearrange(
                                "(k p) c -> p k c", p=128
                            ),
                        )
                    gstats = work.tile(
                        [128, 16], f32, name=f"gstats{i}", tag=f"gstats{i}"
                    )
                    for g in range(G):
                        eng = nc.sync if g % 2 == 0 else nc.scalar
                        eng.dma_start(
                            s_g[g][:, 4:8, :],
                            agob[i][
                                g * BROWS : g * BROWS + HR, :
                            ].rearrange("(k p) c -> p k c", p=128),
                        )
                    for g in range(G):
                        eng = nc.scalar if g % 2 == 0 else nc.sync
                        eng.dma_start(
                            gstats[:, 2 * g : 2 * g + 2].bitcast(bf16),
                            agob[i][g * BROWS + HR : (g + 1) * BROWS, :],
                        )
                else:
                    # ---- final layer: stats-only AllGather ----------------
                    nc.vector.tensor_add(st2o[:], st4[:, 0:2], st4[:, 2:4])
                    nc.scalar.dma_start(agi2[:], st2o[:].bitcast(bf16))
                    nc.gpsimd.collective_compute(
                        "AllGather",
                        mybir.AluOpType.bypass,
                        replica_groups=rg,
                        ins=[agi2.opt()],
                        outs=[ago2.opt()],
                    )
                    gs2 = work.tile([128, 16], f32, name="gs2", tag="gs2")
                    for g in range(G):
                        eng = nc.scalar if g % 2 == 0 else nc.sync
                        eng.dma_start(
                            gs2[:, 2 * g : 2 * g + 2].bitcast(bf16),
                            ago2[g * 4 : g * 4 + 4, :],
                        )
                    aco2, cco2 = _bn_combine(i, gs2, "o")
                    # out^T = (diag(a2) Wout)^T @ zb + (c2 Wout + bout)^T x 1
                    waout = work.tile([128, DOUT], bf16, name="waout", tag="waout")
                    nc.scalar.activation(
                        waout[:], wout_sb[:], AF.Copy, scale=aco2[:]
                    )
                    prow = psS.tile([1, DOUT], f32, name="prow", tag="psS")
                    nc.tensor.matmul(prow[:], cco2[:], wout_sb[:],
                                     start=True, stop=True)
                    orow = work.tile([1, DOUT], bf16, name="orow", tag="orow")
                    nc.vector.tensor_add(orow[:], prow[:], boutb_sb[:])
                    osb = work.tile([DOUT, R], f32, name="osb", tag="osb")
                    for nch in range(R // 512):
                        lo = nch * 512
                        po = psS.tile([DOUT, 512], f32, name="po", tag="psS")
                        nc.tensor.matmul(
                            po[:], orow[:], ones_sb[:],
                            start=True, stop=False,
                        )
                        nc.tensor.matmul(
                            po[:], waout[:], zb[:, lo : lo + 512],
                            start=False, stop=True,
                        )
                        nc.vector.tensor_copy(osb[:, lo : lo + 512], po[:])
                        nc.sync.dma_start(
                            out[:, lo : lo + 512], osb[:, lo : lo + 512]
                        )

    nc.compile()
    return nc


def _get_module():
    if "nc" not in _cache:
        _cache["nc"] = _build_module()
    return _cache["nc"]


def _prep_inputs(inputs):
    """Host-side sharding / layout prep (transpose + cast + slice + degrees)."""
    x = np.asarray(inputs["x"], np.float32)
    adj = np.asarray(inputs["adj"], np.float32)
    bias = np.stack(
        [np.asarray(inputs[f"b{i}"], np.float32) for i in range(NLAYERS)]
    ).reshape(NLAYERS, 128, 1)
    gamma = np.stack(
        [np.asarray(inputs[f"g{i}"], np.float32) for i in range(NLAYERS)]
    ).reshape(NLAYERS, 128, 1)
    beta = np.stack(
        [np.asarray(inputs[f"be{i}"], np.float32) for i in range(NLAYERS)]
    ).reshape(NLAYERS, 128, 1)
    common = {
        "xn": x.astype(BF16),
        "w0": np.asarray(inputs["W0"], np.float32).astype(BF16),
        "w1": np.asarray(inputs["W1"], np.float32).astype(BF16),
        "w2": np.asarray(inputs["W2"], np.float32).astype(BF16),
        "wout": np.asarray(inputs["Wout"], np.float32),
        "boutb": np.asarray(inputs["bout"], np.float32).reshape(1, DOUT),
        "idn": np.eye(128, dtype=np.float32).astype(BF16),
        "biasd": bias,
        "gammad": gamma,
        "betad": beta,
    }
    deg = adj.sum(axis=1)                                          # [N]
    in_maps = []
    for c in range(NC):
        rows = slice(c * R, (c + 1) * R)
        adjt_c = np.ascontiguousarray(adj[rows, :].astype(FP8).T)  # [N, R]
        d_c = deg[rows].reshape(1, R).astype(BF16)
        in_maps.append({"adjt": adjt_c, "dd": d_c, **common})
    return in_maps


def run(inputs, trace=False):
    from concourse.bass_utils import run_bass_kernel_spmd

    nc = _get_module()
    in_maps = _prep_inputs(inputs)
    res = run_bass_kernel_spmd(
        nc, in_maps, core_ids=list(range(NC)), trace=trace
    )
    out = np.concatenate(
        [res.results[c]["out"].T for c in range(NC)], axis=0
    ).astype(np.float32)
    return out, res


def kernel(**inputs):
    out, _ = run(inputs, trace=False)
    return out


# revision 12
# speedup vs baseline: 1.0475x; 1.0475x over previous
"""Trainium2 Bass kernel for a 3-layer GCN (KnowledgeGraphGNN).

Reference (per layer i):  h = BN_i(relu(adj @ (h @ W_i) + b_i)),  then
out = h @ Wout + bout.

Sharding: nodes row-partitioned over 8 cores.  Each core keeps its adj^T
column block [N, R] resident in SBUF as fp8 (adj is 0/1 -> exact) and the
full "stationary" activation matrix [N, 128] in bf16 k-tiles.  The
aggregation matmul runs in transposed space: P^T [128, R] = S^T @ adjT_c,
64 k-tiles accumulated on the PE (N=512 moving slices).

Layer 0 uses associativity: adj @ (x W0) = (adj @ x) @ W0, so the
stationaries are x's own k-tiles (DMA'd in natural layout, no on-chip
build) and W0 is applied post-aggregation like the BN-folded W_i of later
layers.  Layer-0 accumulation is group-major so the PE tracks the
streaming adj DMA.

Collective structure: each hidden layer's AllGather of the raw post-ReLU
activations R_i is SPLIT in two half-payload AllGathers (chunk 0 rows,
then chunk 1 rows + packed BN partial sums).  AG_a's wire time hides
under the producer's chunk-1 matmuls; AG_b's hides under the consumer's
first-half k-tile accumulation (the 32 k-tiles AG_a already delivered).
The BN affine is folded into the next layer algebraically:

    h_{i+1} = adj @ (BN_i(R_i) @ W) = (adj @ R_i) @ diag(a_i) W  +  d x r_i

with a_i = gamma_i/sigma_i, r_i = (beta_i - mu_i a_i) @ W, and d = adj @ 1
the degree vector.  The rank-1 d x r term is seeded into PSUM with a K=1
outer-product matmul before the W matmuls accumulate on top.  The final
layer needs only a tiny stats AllGather; its output is produced
transposed ([DOUT, R] via diag(a2)Wout-matmuls against the resident z^T)
and untransposed host-side.
"""

import numpy as np
import ml_dtypes

BF16 = ml_dtypes.bfloat16
FP8 = ml_dtypes.float8_e4m3

N = 8192          # nodes
DH = 128          # hidden dim (= partition count)
DOUT = 64
NC = 8            # cores
R = N // NC       # rows per core = 1024
KT = N // 128     # contraction k-tiles = 64
G = 8             # k-tile groups (8 tiles each)
RT = R // 128     # node tiles per core = 8
NLAYERS = 3
EPS = 1e-5
HR = 512          # half-payload rows
BROWS = HR + 4    # AG_b rows: chunk-1 rows + 4 rows (=1KB) stats

_cache = {}


def _build_module():
    from concourse import bacc, tile
    import concourse.mybir as mybir

    f32 = mybir.dt.float32
    bf16 = mybir.dt.bfloat16
    fp8 = mybir.dt.float8e4
    AF = mybir.ActivationFunctionType

    nc = bacc.Bacc(None, target_bir_lowering=False, num_devices=NC)

    # ---- kernel I/O --------------------------------------------------------
    adjt = nc.dram_tensor("adjt", [N, R], fp8, kind="ExternalInput")
    xn = nc.dram_tensor("xn", [N, 128], bf16, kind="ExternalInput")
    w0 = nc.dram_tensor("w0", [128, 128], bf16, kind="ExternalInput")
    w1 = nc.dram_tensor("w1", [128, 128], bf16, kind="ExternalInput")
    w2 = nc.dram_tensor("w2", [128, 128], bf16, kind="ExternalInput")
    wout = nc.dram_tensor("wout", [128, DOUT], f32, kind="ExternalInput")
    boutb = nc.dram_tensor("boutb", [1, DOUT], f32, kind="ExternalInput")
    dd = nc.dram_tensor("dd", [1, R], bf16, kind="ExternalInput")
    idn = nc.dram_tensor("idn", [128, 128], bf16, kind="ExternalInput")
    biasd = nc.dram_tensor("biasd", [NLAYERS, 128, 1], f32, kind="ExternalInput")
    gammad = nc.dram_tensor("gammad", [NLAYERS, 128, 1], f32, kind="ExternalInput")
    betad = nc.dram_tensor("betad", [NLAYERS, 128, 1], f32, kind="ExternalInput")
    out = nc.dram_tensor("out", [DOUT, R], f32, kind="ExternalOutput")

    rg = [list(range(NC))]
    wdram = (w0, w1, w2)

    with tile.TileContext(nc) as tc:
        with (
            tc.tile_pool(name="const", bufs=1) as const,
            tc.tile_pool(name="adjp", bufs=1) as adjp,
            tc.tile_pool(name="sp", bufs=1) as sp,
            tc.tile_pool(name="work", bufs=1) as work,
            tc.tile_pool(name="psA", bufs=1, space="PSUM") as psA,
            tc.tile_pool(name="psH", bufs=1, space="PSUM") as psH,
            tc.tile_pool(name="psS", bufs=2, space="PSUM") as psS,
            tc.tile_pool(name="psT", bufs=2, space="PSUM") as psT,
            tc.tile_pool(name="dram", bufs=1, space="DRAM") as dram,
        ):
            # ---- constants (scalar engine issues these tiny DMAs) ----------
            w_sb = []
            for i in range(NLAYERS):
                t = const.tile([128, 128], bf16, name=f"w{i}_sb", tag=f"w{i}_sb")
                nc.scalar.dma_start(t[:], wdram[i][:])
                w_sb.append(t)
            wout_sb = const.tile([128, DOUT], f32, name="wout_sb")
            nc.scalar.dma_start(wout_sb[:], wout[:])
            boutb_sb = const.tile([1, DOUT], f32, name="boutb_sb")
            nc.scalar.dma_start(boutb_sb[:], boutb[:])
            w0f = const.tile([128, 128], f32, name="w0f")
            nc.vector.tensor_copy(w0f[:], w_sb[0][:])
            ones_sb = const.tile([1, 512], bf16, name="ones_sb")
            nc.vector.memset(ones_sb[:], 1.0)
            d_sb = const.tile([1, R], bf16, name="d_sb")
            nc.scalar.dma_start(d_sb[:], dd[:])
            idn_sb = const.tile([128, 128], bf16, name="idn_sb")
            nc.scalar.dma_start(idn_sb[:], idn[:])
            bias_sb = const.tile([128, NLAYERS], f32, name="bias_sb")
            gamma_sb = const.tile([128, NLAYERS], f32, name="gamma_sb")
            beta_sb = const.tile([128, NLAYERS], f32, name="beta_sb")
            for i in range(NLAYERS):
                nc.scalar.dma_start(bias_sb[:, i : i + 1], biasd[i])
                nc.scalar.dma_start(gamma_sb[:, i : i + 1], gammad[i])
                nc.scalar.dma_start(beta_sb[:, i : i + 1], betad[i])

            # ---- stationary activation tiles (8 groups of 8 k-tiles) -------
            # Layer 0 stationaries are x's own k-tiles, DMA'd directly in
            # natural [node, feat] layout.
            s_g = [
                sp.tile([128, 8, 128], bf16, name=f"s_{g}", tag=f"s_{g}")
                for g in range(G)
            ]
            for g in range(G):
                nc.sync.dma_start(
                    s_g[g][:],
                    xn[g * 1024 : (g + 1) * 1024, :].rearrange(
                        "(k p) c -> p k c", p=128
                    ),
                )

            # ---- adj^T resident in SBUF: 8 group tiles, 1 DMA each ---------
            adj_g = []
            for g in range(G):
                t = adjp.tile([128, 8, R], fp8, name=f"adj_{g}", tag=f"adj_{g}")
                src = adjt[g * 1024 : (g + 1) * 1024, :].rearrange(
                    "(k p) c -> p k c", p=128
                )
                nc.gpsimd.dma_start(t[:], src)
                adj_g.append(t)

            def s_tile(k):
                g, sub = divmod(k, 8)
                return s_g[g][:, sub, :]

            def adj_mv(k, lo, size):
                g, sub = divmod(k, 8)
                return adj_g[g][:, sub, lo : lo + size]

            # tiny dummy AllGather, triggered at program start (no deps):
            # absorbs the first-collective ncfw/mesh setup cost off the
            # real layer-0 AllGather; its data drains under the input-DMA
            # window.
            agiw = dram.tile([64, 128], bf16, name="agiw", tag="agiw")
            agow = dram.tile([NC * 64, 128], bf16, name="agow", tag="agow",
                             addr_space="Shared")
            nc.gpsimd.collective_compute(
                "AllGather", mybir.AluOpType.bypass, replica_groups=rg,
                ins=[agiw.opt()], outs=[agow.opt()],
            )

            # per-layer DRAM comm tiles
            AGROWS = R + 4
            agi = [
                dram.tile([AGROWS, 128], bf16, name=f"agi{i}", tag=f"agi{i}")
                for i in range(2)
            ]
            ago = [
                dram.tile(
                    [NC * AGROWS, 128], bf16, name=f"ago{i}", tag=f"ago{i}",
                    addr_space="Shared",
                )
                for i in range(2)
            ]
            agi2 = dram.tile([4, 128], bf16, name="agi2", tag="agi2")
            ago2 = dram.tile([32, 128], bf16, name="ago2", tag="ago2",
                             addr_space="Shared")

            gstats = None  # SBUF tile holding the 8 gathered stat blocks
            _wa = [None]
            _rrow = [None]

            def _bn_combine(i, gst, pref):
                # reduce the 8 gathered per-core stat pairs -> mu/E[x^2],
                # then inv = 1/sqrt(var+eps) with one Newton-rsqrt step to
                # scrub the HW sqrt/recip table error.
                gsc = work.tile([128, 8], f32, name=f"gsc{pref}", tag=f"gsc{pref}")
                st2 = work.tile([128, 2], f32, name=f"st2{pref}", tag=f"st2{pref}")
                nc.vector.tensor_add(gsc[:], gst[:, 0:8], gst[:, 8:16])
                nc.vector.tensor_add(gsc[:, 0:4], gsc[:, 0:4], gsc[:, 4:8])
                nc.vector.tensor_add(st2[:], gsc[:, 0:2], gsc[:, 2:4])
                mu = work.tile([128, 1], f32, name=f"mu{pref}", tag=f"mu{pref}")
                ex2 = work.tile([128, 1], f32, name=f"ex2{pref}", tag=f"ex2{pref}")
                var = work.tile([128, 1], f32, name=f"var{pref}", tag=f"var{pref}")
                sd = work.tile([128, 1], f32, name=f"sd{pref}", tag=f"sd{pref}")
                y0 = work.tile([128, 1], f32, name=f"y0{pref}", tag=f"y0{pref}")
                yy = work.tile([128, 1], f32, name=f"yy{pref}", tag=f"yy{pref}")
                hvy = work.tile([128, 1], f32, name=f"hv{pref}", tag=f"hv{pref}")
                inv = work.tile([128, 1], f32, name=f"inv{pref}", tag=f"inv{pref}")
                aco = work.tile([128, 1], f32, name=f"aco{pref}", tag=f"aco{pref}")
                cco = work.tile([128, 1], f32, name=f"cco{pref}", tag=f"cco{pref}")
                nc.vector.tensor_scalar_mul(mu[:], st2[:, 0:1], 1.0 / N)
                nc.vector.tensor_scalar_mul(ex2[:], st2[:, 1:2], 1.0 / N)
                nc.vector.tensor_mul(var[:], mu[:], mu[:])
                nc.vector.tensor_sub(var[:], ex2[:], var[:])
                nc.vector.tensor_scalar_add(var[:], var[:], EPS)
                nc.scalar.sqrt(sd[:], var[:])
                nc.vector.reciprocal(y0[:], sd[:])
                # Newton: inv = y0 * (1.5 - 0.5*var*y0^2)
                nc.vector.tensor_mul(yy[:], y0[:], y0[:])
                nc.vector.tensor_mul(yy[:], yy[:], var[:])
                nc.vector.tensor_scalar_mul(yy[:], yy[:], -0.5)
                nc.vector.tensor_scalar_add(hvy[:], yy[:], 1.5)
                nc.vector.tensor_mul(inv[:], y0[:], hvy[:])
                nc.vector.tensor_mul(aco[:], gamma_sb[:, i : i + 1], inv[:])
                nc.vector.tensor_mul(cco[:], mu[:], aco[:])
                nc.vector.tensor_sub(cco[:], beta_sb[:, i : i + 1], cco[:])
                return aco, cco

            def _emit_bn_fold(i):
                # combine gathered stats of layer i-1; build Wa = diag(a) W_i
                # and r = (beta - mu a) @ W_i
                aco, cco = _bn_combine(i - 1, gstats, f"f{i}")
                ccb = work.tile([128, 1], bf16, name=f"ccb{i}", tag=f"ccb{i}")
                nc.vector.tensor_copy(ccb[:], cco[:])
                wa = work.tile([128, 128], f32, name="wa", tag="wa")
                nc.scalar.activation(wa[:], w_sb[i][:], AF.Copy, scale=aco[:])
                pr = psS.tile([1, 128], f32, name="pr", tag="psS")
                nc.tensor.matmul(pr[:], ccb[:], w_sb[i][:])
                rrow = work.tile([1, 128], bf16, name="rrow", tag="rrow")
                nc.vector.tensor_copy(rrow[:], pr[:])
                _wa[0] = wa
                _rrow[0] = rrow

            for i in range(NLAYERS):
                zb = work.tile([128, R], bf16, name="zb", tag="zb")
                sq = work.tile([128, R], f32, name="sq", tag="sq")
                st4 = work.tile([128, 4], f32, name="st4", tag="st4")
                if i < NLAYERS - 1:
                    rnat = work.tile([128, 8, 128], bf16, name="rnat", tag="rnat")

                def _bc_chunk(c):
                    # relu+bias (+stats) for 512-chunk c, then its transposes
                    lo = c * 512
                    nc.scalar.activation(
                        zb[:, lo : lo + 512],
                        ph[:, lo : lo + 512],
                        AF.Relu,
                        bias=bias_sb[:, i : i + 1],
                        scale=1.0,
                        accum_out=st4[:, 2 * c : 2 * c + 1],
                    )
                    nc.scalar.activation(
                        sq[:, lo : lo + 512],
                        zb[:, lo : lo + 512],
                        AF.Square,
                        accum_out=st4[:, 2 * c + 1 : 2 * c + 2],
                    )
                    if i < NLAYERS - 1:
                        for t in range(4 * c, 4 * c + 4):
                            ptp = psT.tile([128, 128], bf16, name="ptp", tag="psT")
                            nc.tensor.transpose(
                                ptp[:], zb[:, t * 128 : (t + 1) * 128], idn_sb[:]
                            )
                            nc.vector.tensor_copy(rnat[:, t, :], ptp[:])
                        nc.sync.dma_start(
                            agi[i][c * 512 : (c + 1) * 512, :].rearrange(
                                "(k p) c -> p k c", p=128
                            ),
                            rnat[:, 4 * c : 4 * c + 4, :],
                        )

                def _finish_chunk(nch):
                    # pm copy, (BN fold), rank-1 seed + W matmul, relu/stats
                    lo = nch * 512
                    nc.vector.tensor_copy(pm[:, lo : lo + 512], pa[:, lo : lo + 512])
                    if i == 0:
                        nc.tensor.matmul(
                            ph[:, lo : lo + 512], w0f[:], pm[:, lo : lo + 512],
                            start=True, stop=True,
                        )
                    else:
                        if nch == 0:
                            _emit_bn_fold(i)
                        nc.tensor.matmul(
                            ph[:, lo : lo + 512], _rrow[0][:],
                            d_sb[:, lo : lo + 512], start=True, stop=False,
                        )
                        nc.tensor.matmul(
                            ph[:, lo : lo + 512], _wa[0][:],
                            pm[:, lo : lo + 512], start=False, stop=True,
                        )
                    _bc_chunk(nch)

                pa = psA.tile([128, R], f32, name="pa", tag="pa")
                pm = work.tile([128, R], f32, name="pm", tag="pm")
                ph = psH.tile([128, R], f32, name="ph", tag="ph")
                st2o = work.tile([128, 2], f32, name="st2o", tag="st2o")

                if i == 0:
                    # group-major over both chunks: the PE consumes each adj
                    # group right after its DMA lands, so layer 0 finishes
                    # ~when the adj stream does.
                    first = [True, True]
                    for g in range(G):
                        for nch in range(2):
                            lo = nch * 512
                            for sub in range(8):
                                k = 8 * g + sub
                                nc.tensor.matmul(
                                    pa[:, lo : lo + 512], s_tile(k),
                                    adj_mv(k, lo, 512),
                                    start=first[nch],
                                    stop=(g == G - 1 and sub == 7),
                                )
                                first[nch] = False
                    for nch in range(2):
                        _finish_chunk(nch)
                else:
                    for nch in range(2):
                        lo = nch * 512
                        for k in range(KT):
                            nc.tensor.matmul(
                                pa[:, lo : lo + 512], s_tile(k),
                                adj_mv(k, lo, 512),
                                start=(k == 0), stop=(k == KT - 1),
                            )
                        _finish_chunk(nch)

                nc.vector.tensor_add(st2o[:], st4[:, 0:2], st4[:, 2:4])

                if i < NLAYERS - 1:
                    nc.scalar.dma_start(
                        agi[i][R : R + 4, :], st2o[:].bitcast(bf16)
                    )
                    nc.gpsimd.collective_compute(
                        "AllGather", mybir.AluOpType.bypass, replica_groups=rg,
                        ins=[agi[i].opt()], outs=[ago[i].opt()],
                    )
                    for g in range(G):
                        eng = nc.sync if g % 2 == 0 else nc.scalar
                        eng.dma_start(
                            s_g[g][:],
                            ago[i][
                                g * AGROWS : g * AGROWS + R, :
                            ].rearrange("(k p) c -> p k c", p=128),
                        )
                    gstats = work.tile(
                        [128, 16], f32, name=f"gstats{i}", tag=f"gstats{i}"
                    )
                    for g in range(G):
                        eng = nc.scalar if g % 2 == 0 else nc.sync
                        eng.dma_start(
                            gstats[:, 2 * g : 2 * g + 2].bitcast(bf16),
                            ago[i][g * AGROWS + R : g * AGROWS + R + 4, :],
                        )
                else:
                    # ---- final layer: stats-only AllGather ----------------
                    nc.scalar.dma_start(agi2[:], st2o[:].bitcast(bf16))
                    nc.gpsimd.collective_compute(
                        "AllGather",
                        mybir.AluOpType.bypass,
                        replica_groups=rg,
                        ins=[agi2.opt()],
                        outs=[ago2.opt()],
                    )
                    gs2 = work.tile([128, 16], f32, name="gs2", tag="gs2")
                    for g in range(G):
                        eng = nc.scalar if g % 2 == 0 else nc.sync
                        eng.dma_start(
                            gs2[:, 2 * g : 2 * g + 2].bitcast(bf16),
                            ago2[g * 4 : g * 4 + 4, :],
                        )
                    aco2, cco2 = _bn_combine(i, gs2, "o")
                    # out^T = (diag(a2) Wout)^T @ zb + (c2 Wout + bout)^T x 1
                    waout = work.tile([128, DOUT], bf16, name="waout", tag="waout")
                    nc.scalar.activation(
                        waout[:], wout_sb[:], AF.Copy, scale=aco2[:]
                    )
                    prow = psS.tile([1, DOUT], f32, name="prow", tag="psS")
                    nc.tensor.matmul(prow[:], cco2[:], wout_sb[:],
                                     start=True, stop=True)
                    orow = work.tile([1, DOUT], bf16, name="orow", tag="orow")
                    nc.vector.tensor_add(orow[:], prow[:], boutb_sb[:])
                    osb = work.tile([DOUT, R], f32, name="osb", tag="osb")
                    for nch in range(R // 512):
                        lo = nch * 512
                        po = psS.tile([DOUT, 512], f32, name="po", tag="psS")
                        nc.tensor.matmul(
                            po[:], orow[:], ones_sb[:],
                            start=True, stop=False,
                        )
                        nc.tensor.matmul(
                            po[:], waout[:], zb[:, lo : lo + 512],
                            start=False, stop=True,
                        )
                        nc.vector.tensor_copy(osb[:, lo : lo + 512], po[:])
                        nc.sync.dma_start(
                            out[:, lo : lo + 512], osb[:, lo : lo + 512]
                        )

    nc.compile()
    return nc


def _get_module():
    if "nc" not in _cache:
        _cache["nc"] = _build_module()
    return _cache["nc"]


def _prep_inputs(inputs):
    """Host-side sharding / layout prep (transpose + cast + slice + degrees)."""
    x = np.asarray(inputs["x"], np.float32)
    adj = np.asarray(inputs["adj"], np.float32)
    bias = np.stack(
        [np.asarray(inputs[f"b{i}"], np.float32) for i in range(NLAYERS)]
    ).reshape(NLAYERS, 128, 1)
    gamma = np.stack(
        [np.asarray(inputs[f"g{i}"], np.float32) for i in range(NLAYERS)]
    ).reshape(NLAYERS, 128, 1)
    beta = np.stack(
        [np.asarray(inputs[f"be{i}"], np.float32) for i in range(NLAYERS)]
    ).reshape(NLAYERS, 128, 1)
    common = {
        "xn": x.astype(BF16),
        "w0": np.asarray(inputs["W0"], np.float32).astype(BF16),
        "w1": np.asarray(inputs["W1"], np.float32).astype(BF16),
        "w2": np.asarray(inputs["W2"], np.float32).astype(BF16),
        "wout": np.asarray(inputs["Wout"], np.float32),
        "boutb": np.asarray(inputs["bout"], np.float32).reshape(1, DOUT),
        "idn": np.eye(128, dtype=np.float32).astype(BF16),
        "biasd": bias,
        "gammad": gamma,
        "betad": beta,
    }
    deg = adj.sum(axis=1)                                          # [N]
    in_maps = []
    for c in range(NC):
        rows = slice(c * R, (c + 1) * R)
        adjt_c = np.ascontiguousarray(adj[rows, :].astype(FP8).T)  # [N, R]
        d_c = deg[rows].reshape(1, R).astype(BF16)
        in_maps.append({"adjt": adjt_c, "dd": d_c, **common})
    return in_maps


def run(inputs, trace=False):
    from concourse.bass_utils import run_bass_kernel_spmd

    nc = _get_module()
    in_maps = _prep_inputs(inputs)
    res = run_bass_kernel_spmd(
        nc, in_maps, core_ids=list(range(NC)), trace=trace
    )
    out = np.concatenate(
        [res.results[c]["out"].T for c in range(NC)], axis=0
    ).astype(np.float32)
    return out, res


def kernel(**inputs):
    out, _ = run(inputs, trace=False)
    return out
